# revision 2
# baseline (speedup 1.0000x reference)
"""Trainium2 Bass kernel for nn_CharacterEmbeddingLayer (v2).

Computation (see reference):
  embed = char_vectors[char_idxs]                       # [B,S,16,64]
  per window w in (2,3,4,5):
      h_w = max_l tanh(conv_w(embed))                   # [B,S,100]
  x = concat(h_w) @ w_proj.T                            # [B,S,128]
  2x highway: x = g*relu(Wt x+bt) + (1-g)*x, g=sigmoid(Wg x+bg)

Device mapping (per core, data-parallel over batch: 8 rows => 3200 tokens):
  - one-hot built on DVE (tensor_scalar is_equal, int16 in, bf16 out, 4x
    mode) from broadcast-DMA'd indices vs a per-partition iota column
  - embeddings in paired layout: e_sb slot p<8 = E0 pair (2p,2p+1)
    [dims of even char on partitions 0:64, odd char on 64:128], built via
    PE matmul cv.T @ one-hot; slots 8..14 = E1 pairs (2s+1,2s+2) built by
    partition-shifted copies on the (otherwise idle) Pool engine
  - conv: every full char-pair of a window position is ONE K=128 matmul
    against E0 or E1 by parity (105 matmuls/tile vs 118 with E0 only);
    odd-w trailing taps are zero-padded half-K units on E0
  - psum groups of 3 positions; drains split between ACT (batch extract
    -> bf16 slab) and DVE (fused max-reduce straight from PSUM); per
    window a slab-tree of wide tensor_max ops folds positions; tanh
    deferred past the max (monotonic)
  - embedding prep for tile i+1 is emitted early inside tile i's body so
    the ACT extracts + Pool E1 copies overlap tile i's conv instead of
    stalling PE at the tile boundary
  - projection + highway on PE/ACT/DVE; output stored feature-major f32
    and transposed on the host.
"""

import sys

sys.path.insert(0, "/opt/trn_rl_repo")

import numpy as np
import ml_dtypes

B, S, W, D = 64, 400, 16, 64
VOCAB, HID, NF = 96, 128, 100
WINDOWS = (2, 3, 4, 5)
N_CORES = 8
TOK_PER_CORE = B * S // N_CORES  # 3200
T = 512  # max tokens per tile (PSUM bank = 512 fp32)
TILES = [(t0, min(T, TOK_PER_CORE - t0)) for t0 in range(0, TOK_PER_CORE, T)]
N_TILES = len(TILES)  # 6x512 + 1x128
GRP = 3  # conv positions per PSUM group tile (3 banks)

_cache = {}

BF16 = ml_dtypes.bfloat16


# ---------------------------------------------------------------- schedule
def build_schedule():
    """Conv decomposition with E0+E1 paired layouts.

    e_sb slots: 0..7 = E0 pair p (chars 2p,2p+1); 8..14 = E1 pair s
    (chars 2s+1,2s+2). Every full pair (l+j, l+j+1) is one K=128 matmul
    against the slot of matching parity; odd-w trailing tap is a
    half-K 'single' unit on E0 (zeros in the unused half).

    units: key -> index; ('pair', w, j) stacks taps [j; j+1],
           ('single', w, h) has tap w-1 in K-half h.
    sched: list of (w, [ops per l]); op = (unit_idx, e_slot).
    """
    units = {}

    def uidx(key):
        if key not in units:
            units[key] = len(units)
        return units[key]

    sched = []
    for w in WINDOWS:
        L = W - w + 1
        wl = []
        for l in range(L):
            ops = []
            for j in range(0, w - 1, 2):
                c = l + j
                slot = c // 2 if c % 2 == 0 else 8 + (c - 1) // 2
                ops.append((uidx(("pair", w, j)), slot))
            if w % 2 == 1:
                c = l + w - 1
                h = c % 2
                slot = c // 2 if h == 0 else (c - 1) // 2  # always E0
                ops.append((uidx(("single", w, h)), slot))
            wl.append(ops)
        sched.append((w, wl))
    return units, sched


def window_groups(L):
    out = []
    l = 0
    while l < L:
        n = min(GRP, L - l)
        out.append((l, n))
        l += n
    return out


# Drain engine per (window index, group index): True = DVE fused reduce,
# False = ACT extract into the slab. Balanced so ACT ~= DVE busy.
DVE_GROUPS = {(3, 0), (3, 1), (3, 2), (3, 3), (2, 0), (2, 1)}


def conv_plan():
    """Emission-ordered conv groups, interleaving heavy-fill (w5/w4) with
    light-fill (w2/w3) windows to hide drain latency in the 2-deep psum
    rotation. Returns list of (wi, gi, l0, n, slab_slot, dve)."""
    units, sched = build_schedule()
    per_w = []
    for wi, (w, wl) in enumerate(sched):
        per_w.append([(wi, gi, l0, n) for gi, (l0, n) in
                      enumerate(window_groups(len(wl)))])
    order = []
    a, b = per_w[3], per_w[0]  # w5 (4 groups), w2 (5 groups)
    for i in range(max(len(a), len(b))):
        if i < len(a):
            order.append(a[i])
        if i < len(b):
            order.append(b[i])
    a, b = per_w[2], per_w[1]  # w4 (5 groups), w3 (5 groups)
    for i in range(max(len(a), len(b))):
        if i < len(a):
            order.append(a[i])
        if i < len(b):
            order.append(b[i])
    # slab slot assignment: per window, in emission order
    base = {}
    off = 0
    for wi in range(4):
        base[wi] = off
        off += sum(1 if (wi, gi) in DVE_GROUPS else n
                   for (wi_, gi, l0, n) in per_w[wi])
    plan = []
    cur = dict((wi, base[wi]) for wi in range(4))
    for (wi, gi, l0, n) in order:
        dve = (wi, gi) in DVE_GROUPS
        plan.append((wi, gi, l0, n, cur[wi], dve))
        cur[wi] += 1 if dve else n
    slab_bounds = {wi: (base[wi], cur[wi]) for wi in range(4)}
    return plan, slab_bounds, off


# ---------------------------------------------------------------- host prep
def prep_weights(char_vectors, filts, w_proj, hw_ws, hw_bs):
    units, _ = build_schedule()
    U = len(units)
    wconv = np.zeros((128, U, 128), np.float32)
    for key, u in units.items():
        kind, w = key[0], key[1]
        f = filts[w].reshape(NF, w, D)  # [100, w, 64]
        if kind == "pair":
            j = key[2]
            wconv[0:64, u, 0:NF] = f[:, j, :].T
            wconv[64:128, u, 0:NF] = f[:, j + 1, :].T
        else:
            h = key[2]
            wconv[64 * h:64 * h + 64, u, 0:NF] = f[:, w - 1, :].T
    wproj = np.zeros((128, 4, 128), np.float32)
    for c in range(4):
        wproj[0:NF, c, :] = w_proj[:, c * NF:(c + 1) * NF].T
    whw = np.zeros((128, 4, 128), np.float32)
    for i, wm in enumerate(hw_ws):  # [t_w0, g_w0, t_w1, g_w1]
        whw[:, i, :] = wm.T
    bias = np.zeros((128, 4), np.float32)
    for i, bv in enumerate(hw_bs):  # [t_b0, g_b0, t_b1, g_b1]
        bias[:, i] = bv
    return {
        "cv": np.ascontiguousarray(char_vectors.astype(BF16)),
        "wconv": np.ascontiguousarray(wconv.astype(BF16)),
        "wproj": np.ascontiguousarray(wproj.astype(BF16)),
        "whw": np.ascontiguousarray(whw.astype(BF16)),
        "bias": np.ascontiguousarray(bias),
    }


# ---------------------------------------------------------------- program
def build_program(n_tiles=N_TILES, repeat=1):
    from concourse import bacc
    import concourse.mybir as mybir
    from concourse.tile import TileContext

    dt = mybir.dt
    AF = mybir.ActivationFunctionType
    AL = mybir.AluOpType
    units, sched = build_schedule()
    U = len(units)
    plan, slab_bounds, n_slab = conv_plan()

    nc = bacc.Bacc("TRN2", target_bir_lowering=False, debug=False, num_devices=N_CORES)

    idx_d = nc.dram_tensor("idx", [16, TOK_PER_CORE], dt.int16, kind="ExternalInput")
    cv_d = nc.dram_tensor("cv", [VOCAB, D], dt.bfloat16, kind="ExternalInput")
    wconv_d = nc.dram_tensor("wconv", [128, U, 128], dt.bfloat16, kind="ExternalInput")
    wproj_d = nc.dram_tensor("wproj", [128, 4, 128], dt.bfloat16, kind="ExternalInput")
    whw_d = nc.dram_tensor("whw", [128, 4, 128], dt.bfloat16, kind="ExternalInput")
    bias_d = nc.dram_tensor("bias", [128, 4], dt.float32, kind="ExternalInput")
    out_d = nc.dram_tensor("out", [128, TOK_PER_CORE], dt.float32, kind="ExternalOutput")

    with TileContext(nc) as tc:
        with (
            tc.tile_pool(name="const", bufs=1) as cpool,
            tc.tile_pool(name="io", bufs=2) as iopool,
            tc.tile_pool(name="work", bufs=2) as wpool,
            tc.tile_pool(name="slab", bufs=1) as spool,
            tc.tile_pool(name="grp_psum", bufs=2, space="PSUM") as gpsum,
            tc.tile_pool(name="vec_psum", bufs=2, space="PSUM") as vpsum,
        ):
            cv_sb = cpool.tile([VOCAB, D], dt.bfloat16)
            nc.sync.dma_start(cv_sb, cv_d.ap())
            wconv_sb = cpool.tile([128, U, 128], dt.bfloat16)
            nc.sync.dma_start(wconv_sb, wconv_d.ap())
            wproj_sb = cpool.tile([128, 4, 128], dt.bfloat16)
            nc.sync.dma_start(wproj_sb, wproj_d.ap())
            whw_sb = cpool.tile([128, 4, 128], dt.bfloat16)
            nc.sync.dma_start(whw_sb, whw_d.ap())
            bias_sb = cpool.tile([128, 4], dt.float32)
            nc.sync.dma_start(bias_sb, bias_d.ap())
            iota_i = cpool.tile([VOCAB, 1], dt.int32)
            nc.gpsimd.iota(iota_i, pattern=[[1, 1]], base=0, channel_multiplier=1)
            iota_sb = cpool.tile([VOCAB, 1], dt.float32)
            nc.vector.tensor_copy(iota_sb, iota_i)

            def prep_esb(ti, first):
                """one-hot + embed matmuls + extracts + Pool E1 copies for
                tile ti; returns the e_sb tile."""
                tok0, Tc = TILES[ti]
                idx_b = iopool.tile([VOCAB, 16, T], dt.int16, tag="idxb")
                for h0 in (0, 8):
                    nc.sync.dma_start(
                        idx_b[:, h0:h0 + 8, 0:Tc],
                        idx_d.ap()[h0:h0 + 8, tok0:tok0 + Tc]
                        .partition_broadcast(VOCAB),
                    )
                oh = iopool.tile([VOCAB, 16, T], dt.bfloat16, tag="oh")
                for h0 in (0, 8):
                    nc.vector.tensor_scalar(
                        oh[:, h0:h0 + 8, 0:Tc], idx_b[:, h0:h0 + 8, 0:Tc],
                        iota_sb[:, :], None, AL.is_equal)
                e_sb = iopool.tile([128, 15, T], dt.bfloat16, tag="esb")
                for r0 in (0, 3, 6):
                    n = min(GRP, 8 - r0)
                    g = gpsum.tile([128, GRP, 512], dt.float32, tag="grp")
                    for i in range(n):
                        p = r0 + i
                        nc.tensor.matmul(g[0:64, i, 0:Tc], cv_sb, oh[:, 2 * p, 0:Tc],
                                         start=True, stop=True, tile_position=(0, 0))
                        nc.tensor.matmul(g[64:128, i, 0:Tc], cv_sb,
                                         oh[:, 2 * p + 1, 0:Tc],
                                         start=True, stop=True, tile_position=(0, 64))
                    nc.scalar.copy(e_sb[:, r0:r0 + n, 0:Tc], g[:, 0:n, 0:Tc])
                # E1 shifted pairs; Pool steady-state, DVE for the first
                # (latency-critical) tile
                eng = nc.vector if first else nc.gpsimd
                eng.tensor_copy(e_sb[0:64, 8:15, 0:Tc], e_sb[64:128, 0:7, 0:Tc])
                eng.tensor_copy(e_sb[64:128, 8:15, 0:Tc], e_sb[0:64, 1:8, 0:Tc])
                return e_sb

            for _rep in range(repeat):
                e_next = prep_esb(0, True)
                for ti in range(n_tiles):
                    tok0, Tc = TILES[ti]
                    e_sb = e_next
                    pw = spool.tile([128, n_slab, T], dt.bfloat16, tag="pw")
                    mfin = wpool.tile([128, 4, T], dt.bfloat16, tag="m4")

                    done = {wi: 0 for wi in range(4)}
                    n_groups = {wi: 0 for wi in range(4)}
                    for (wi, gi, l0, n, slot, dve) in plan:
                        n_groups[wi] += 1
                    prep_emitted = False

                    for pi, (wi, gi, l0, n, slot, dve) in enumerate(plan):
                        w, wl = sched[wi]
                        g = gpsum.tile([128, GRP, 512], dt.float32, tag="grp")
                        for li in range(n):
                            ops = wl[l0 + li]
                            for oi, (u, sl) in enumerate(ops):
                                nc.tensor.matmul(
                                    g[:, li, 0:Tc], wconv_sb[:, u, :],
                                    e_sb[:, sl, 0:Tc],
                                    start=(oi == 0), stop=(oi == len(ops) - 1),
                                )
                        if dve:
                            if n == 1:
                                nc.vector.tensor_copy(pw[:, slot, 0:Tc],
                                                      g[:, 0, 0:Tc])
                            else:
                                nc.vector.tensor_reduce(
                                    pw[:, slot, 0:Tc],
                                    g[:, 0:n, 0:Tc].rearrange("p g t -> p t g"),
                                    axis=mybir.AxisListType.X, op=AL.max)
                        else:
                            nc.scalar.copy(pw[:, slot:slot + n, 0:Tc],
                                           g[:, 0:n, 0:Tc])
                        done[wi] += 1
                        # slab-tree fold once this window's groups all drained
                        if done[wi] == n_groups[wi]:
                            lo, hi = slab_bounds[wi]
                            K = hi - lo
                            while K > 2:
                                h = K // 2
                                nc.vector.tensor_max(
                                    pw[:, lo:lo + h, 0:Tc],
                                    pw[:, lo:lo + h, 0:Tc],
                                    pw[:, lo + K - h:lo + K, 0:Tc])
                                K = K - h
                            nc.vector.tensor_max(mfin[:, wi, 0:Tc],
                                                 pw[:, lo, 0:Tc],
                                                 pw[:, lo + 1, 0:Tc])
                        # emit next tile's embedding prep early so its
                        # extracts + E1 overlap this tile's conv
                        if pi == 3 and ti + 1 < n_tiles:
                            e_next = prep_esb(ti + 1, False)
                            prep_emitted = True
                    if not prep_emitted and ti + 1 < n_tiles:
                        e_next = prep_esb(ti + 1, False)

                    th = wpool.tile([128, 4, T], dt.bfloat16, tag="th")
                    nc.scalar.activation(th[:, :, 0:Tc], mfin[:, :, 0:Tc], AF.Tanh)

                    # ---- projection
                    x_ps = vpsum.tile([128, 512], dt.float32, tag="vec")
                    for c in range(4):
                        nc.tensor.matmul(x_ps[:, 0:Tc], wproj_sb[:, c, :],
                                         th[:, c, 0:Tc], start=(c == 0), stop=(c == 3))
                    xs = wpool.tile([128, T], dt.bfloat16, tag="xs")
                    nc.scalar.copy(xs[:, 0:Tc], x_ps[:, 0:Tc])

                    # ---- highway x2
                    for hl in range(2):
                        t_ps = vpsum.tile([128, 512], dt.float32, tag="vec")
                        g_ps = vpsum.tile([128, 512], dt.float32, tag="vec")
                        nc.tensor.matmul(t_ps[:, 0:Tc], whw_sb[:, 2 * hl, :],
                                         xs[:, 0:Tc], start=True, stop=True)
                        nc.tensor.matmul(g_ps[:, 0:Tc], whw_sb[:, 2 * hl + 1, :],
                                         xs[:, 0:Tc], start=True, stop=True)
                        tt = wpool.tile([128, T], dt.bfloat16, tag="tt")
                        gg = wpool.tile([128, T], dt.bfloat16, tag="gg")
                        nc.scalar.activation(tt[:, 0:Tc], t_ps[:, 0:Tc], AF.Relu,
                                             bias=bias_sb[:, 2 * hl:2 * hl + 1],
                                             scale=1.0)
                        nc.scalar.activation(gg[:, 0:Tc], g_ps[:, 0:Tc], AF.Sigmoid,
                                             bias=bias_sb[:, 2 * hl + 1:2 * hl + 2],
                                             scale=1.0)
                        dd = wpool.tile([128, T], dt.bfloat16, tag="dd")
                        gd = wpool.tile([128, T], dt.bfloat16, tag="gd")
                        nc.vector.tensor_sub(dd[:, 0:Tc], tt[:, 0:Tc], xs[:, 0:Tc])
                        nc.vector.tensor_mul(gd[:, 0:Tc], gg[:, 0:Tc], dd[:, 0:Tc])
                        if hl == 0:
                            xs2 = wpool.tile([128, T], dt.bfloat16, tag="xs")
                            nc.vector.tensor_add(xs2[:, 0:Tc], xs[:, 0:Tc],
                                                 gd[:, 0:Tc])
                            xs = xs2
                        else:
                            xf = wpool.tile([128, T], dt.float32, tag="xf")
                            nc.vector.tensor_add(xf[:, 0:Tc], xs[:, 0:Tc],
                                                 gd[:, 0:Tc])
                            nc.sync.dma_start(out_d.ap()[:, tok0:tok0 + Tc],
                                              xf[:, 0:Tc])

    nc.compile()
    return nc


# ---------------------------------------------------------------- runner
def _make_sharded(nc):
    import jax
    from jax.sharding import Mesh, PartitionSpec
    from jax.experimental.shard_map import shard_map
    from concourse import bass2jax, mybir

    bass2jax.install_neuronx_cc_hook()
    partition_name = nc.partition_id_tensor.name if nc.partition_id_tensor else None
    in_names, out_names, out_avals = [], [], []
    for alloc in nc.m.functions[0].allocations:
        if not isinstance(alloc, mybir.MemoryLocationSet):
            continue
        name = alloc.memorylocations[0].name
        if alloc.kind == "ExternalInput":
            if name != partition_name:
                in_names.append(name)
        elif alloc.kind == "ExternalOutput":
            out_names.append(name)
            out_avals.append(
                jax.core.ShapedArray(tuple(alloc.tensor_shape), mybir.dt.np(alloc.dtype))
            )
    n_params = len(in_names)
    all_in_names = in_names + out_names
    if partition_name is not None:
        all_in_names = all_in_names + [partition_name]

    def _body(*args):
        operands = list(args)
        if partition_name is not None:
            operands.append(bass2jax.partition_id_tensor())
        outs = bass2jax._bass_exec_p.bind(
            *operands,
            out_avals=tuple(out_avals),
            in_names=tuple(all_in_names),
            out_names=tuple(out_names),
            lowering_input_output_aliases=(),
            sim_require_finite=True,
            sim_require_nnan=True,
            nc=nc,
        )
        return tuple(outs)

    devices = jax.devices()[:N_CORES]
    mesh = Mesh(np.asarray(devices), ("core",))
    n_outs = len(out_names)
    in_specs = (PartitionSpec("core"),) * (n_params + n_outs)
    out_specs = (PartitionSpec("core"),) * n_outs
    fn = jax.jit(
        shard_map(_body, mesh=mesh, in_specs=in_specs, out_specs=out_specs,
                  check_rep=False),
        keep_unused=True,
    )
    meta = {"in_names": in_names, "out_names": out_names, "out_avals": out_avals,
            "n_params": n_params}
    return fn, meta


def _get_runner():
    if "runner" not in _cache:
        nc = build_program()
        _cache["nc"] = nc
        _cache["runner"] = _make_sharded(nc)
    return _cache["runner"]


def _concat_inputs(in_maps, meta):
    concat_in = [
        np.concatenate([in_maps[c][name] for c in range(N_CORES)], axis=0)
        for name in meta["in_names"]
    ]
    concat_zeros = [
        np.zeros((N_CORES * a.shape[0], *a.shape[1:]), a.dtype)
        for a in meta["out_avals"]
    ]
    return concat_in, concat_zeros


def make_in_maps(char_idxs, char_vectors, filt2, filt3, filt4, filt5, w_proj,
                 t_w0, t_b0, t_w1, t_b1, g_w0, g_b0, g_w1, g_b1):
    wts = prep_weights(
        np.asarray(char_vectors, np.float32),
        {2: np.asarray(filt2, np.float32), 3: np.asarray(filt3, np.float32),
         4: np.asarray(filt4, np.float32), 5: np.asarray(filt5, np.float32)},
        np.asarray(w_proj, np.float32),
        [np.asarray(t_w0, np.float32), np.asarray(g_w0, np.float32),
         np.asarray(t_w1, np.float32), np.asarray(g_w1, np.float32)],
        [np.asarray(t_b0, np.float32), np.asarray(g_b0, np.float32),
         np.asarray(t_b1, np.float32), np.asarray(g_b1, np.float32)],
    )
    idx = np.asarray(char_idxs)
    assert idx.shape == (B, S, W)
    rows_per_core = B // N_CORES
    in_maps = []
    for c in range(N_CORES):
        m = dict(wts)
        m["idx"] = np.ascontiguousarray(
            idx[c * rows_per_core:(c + 1) * rows_per_core]
            .reshape(TOK_PER_CORE, 16).T.astype(np.int16)
        )
        in_maps.append(m)
    return in_maps


def kernel(**inputs) -> np.ndarray:
    in_maps = make_in_maps(**inputs)
    sharded, meta = _get_runner()
    concat_in, concat_zeros = _concat_inputs(in_maps, meta)
    out_arrs = sharded(*concat_in, *concat_zeros)
    out = np.asarray(out_arrs[0])  # [8*128, 3200]
    rows_per_core = B // N_CORES
    parts = []
    for c in range(N_CORES):
        oc = out[c * 128:(c + 1) * 128]  # [128, 3200]
        parts.append(oc.T.reshape(rows_per_core, S, HID))
    return np.ascontiguousarray(np.concatenate(parts, axis=0))


def time_kernel(inputs, repeat=(8, 25), reps=20):
    """Per-pass exec time from the slope between two repeat factors."""
    import time
    import jax
    from jax.sharding import Mesh, PartitionSpec, NamedSharding

    in_maps = make_in_maps(**inputs)
    sharded, meta = _get_runner()
    concat_in, concat_zeros = _concat_inputs(in_maps, meta)
    mesh = Mesh(np.asarray(jax.devices()[:N_CORES]), ("core",))
    shd = NamedSharding(mesh, PartitionSpec("core"))
    d_in = [jax.device_put(a, shd) for a in concat_in]
    d_zero = [jax.device_put(a, shd) for a in concat_zeros]

    r1, r2 = repeat
    fns = []
    for r in (r1, r2):
        key = ("rep", r)
        if key not in _cache:
            nc_r = build_program(repeat=r)
            _cache[key] = _make_sharded(nc_r)
        fns.append(_cache[key][0])
    fn_1, fn_2 = fns

    def timed(fn, args):
        t0 = time.perf_counter()
        out = fn(*args)
        jax.block_until_ready(out)
        return time.perf_counter() - t0

    timed(fn_1, (*d_in, *d_zero))
    timed(fn_2, (*d_in, *d_zero))
    diffs, t1s = [], []
    for _ in range(reps):
        a = timed(fn_1, (*d_in, *d_zero))
        b = timed(fn_2, (*d_in, *d_zero))
        t1s.append(a)
        diffs.append(b - a)
    diffs.sort()
    t1s.sort()
    med = diffs[len(diffs) // 2]
    per_pass = med / (r2 - r1)
    return per_pass * 1e9, t1s[len(t1s) // 2] * 1e9, med * 1e9


# revision 5
# speedup vs baseline: 8.4237x; 8.4237x over previous
"""Trainium2 Bass kernel for nn_CharacterEmbeddingLayer (v2).

Computation (see reference):
  embed = char_vectors[char_idxs]                       # [B,S,16,64]
  per window w in (2,3,4,5):
      h_w = max_l tanh(conv_w(embed))                   # [B,S,100]
  x = concat(h_w) @ w_proj.T                            # [B,S,128]
  2x highway: x = g*relu(Wt x+bt) + (1-g)*x, g=sigmoid(Wg x+bg)

Device mapping (per core, data-parallel over batch: 8 rows => 3200 tokens):
  - one-hot built on DVE (tensor_scalar is_equal, int16 in, bf16 out, 4x
    mode) from broadcast-DMA'd indices vs a per-partition iota column
  - embeddings in paired layout: e_sb slot p<8 = E0 pair (2p,2p+1)
    [dims of even char on partitions 0:64, odd char on 64:128], built via
    PE matmul cv.T @ one-hot; slots 8..14 = E1 pairs (2s+1,2s+2) built by
    partition-shifted copies on the (otherwise idle) Pool engine
  - conv: every full char-pair of a window position is ONE K=128 matmul
    against E0 or E1 by parity (105 matmuls/tile vs 118 with E0 only);
    odd-w trailing taps are zero-padded half-K units on E0
  - psum groups of 3 positions; drains split between ACT (batch extract
    -> bf16 slab) and DVE (fused max-reduce straight from PSUM); per
    window a slab-tree of wide tensor_max ops folds positions; tanh
    deferred past the max (monotonic)
  - embedding prep for tile i+1 is emitted early inside tile i's body so
    the ACT extracts + Pool E1 copies overlap tile i's conv instead of
    stalling PE at the tile boundary
  - projection + highway on PE/ACT/DVE; output stored feature-major f32
    and transposed on the host.
"""

import sys

sys.path.insert(0, "/opt/trn_rl_repo")

import numpy as np
import ml_dtypes

B, S, W, D = 64, 400, 16, 64
VOCAB, HID, NF = 96, 128, 100
WINDOWS = (2, 3, 4, 5)
N_CORES = 8
TOK_PER_CORE = B * S // N_CORES  # 3200
T = 512  # max tokens per tile (PSUM bank = 512 fp32)
TILES = [(t0, min(T, TOK_PER_CORE - t0)) for t0 in range(0, TOK_PER_CORE, T)]
N_TILES = len(TILES)  # 6x512 + 1x128
GRP = 3  # conv positions per PSUM group tile (3 banks)

_cache = {}

BF16 = ml_dtypes.bfloat16


# ---------------------------------------------------------------- schedule
def build_schedule():
    """Conv decomposition with E0+E1 paired layouts.

    e_sb slots: 0..7 = E0 pair p (chars 2p,2p+1); 8..14 = E1 pair s
    (chars 2s+1,2s+2). Every full pair (l+j, l+j+1) is one K=128 matmul
    against the slot of matching parity; odd-w trailing tap is a
    half-K 'single' unit on E0 (zeros in the unused half).

    units: key -> index; ('pair', w, j) stacks taps [j; j+1],
           ('single', w, h) has tap w-1 in K-half h.
    sched: list of (w, [ops per l]); op = (unit_idx, e_slot).
    """
    units = {}

    def uidx(key):
        if key not in units:
            units[key] = len(units)
        return units[key]

    sched = []
    for w in WINDOWS:
        L = W - w + 1
        wl = []
        for l in range(L):
            ops = []
            for j in range(0, w - 1, 2):
                c = l + j
                slot = c // 2 if c % 2 == 0 else 8 + (c - 1) // 2
                ops.append((uidx(("pair", w, j)), slot))
            if w % 2 == 1:
                c = l + w - 1
                h = c % 2
                slot = c // 2 if h == 0 else (c - 1) // 2  # always E0
                ops.append((uidx(("single", w, h)), slot))
            wl.append(ops)
        sched.append((w, wl))
    return units, sched


def window_groups(L):
    out = []
    l = 0
    while l < L:
        n = min(GRP, L - l)
        out.append((l, n))
        l += n
    return out


# Drain engine per (window index, group index): True = DVE fused reduce,
# False = ACT extract + DVE fold. Balanced so ACT ~= DVE busy.
DVE_GROUPS = {(3, 0), (3, 1), (3, 2), (3, 3), (2, 0), (2, 1)}


def conv_plan():
    """Emission-ordered conv groups, interleaving heavy-fill (w5/w4) with
    light-fill (w2/w3) windows to hide drain latency in the 2-deep psum
    rotation. Returns list of (wi, gi, l0, n, partial_slot, dve) and the
    per-window partial slot ranges."""
    units, sched = build_schedule()
    per_w = []
    for wi, (w, wl) in enumerate(sched):
        per_w.append([(wi, gi, l0, n) for gi, (l0, n) in
                      enumerate(window_groups(len(wl)))])
    order = []
    a, b = per_w[3], per_w[0]  # w5 (4 groups), w2 (5 groups)
    for i in range(max(len(a), len(b))):
        if i < len(a):
            order.append(a[i])
        if i < len(b):
            order.append(b[i])
    a, b = per_w[2], per_w[1]  # w4 (5 groups), w3 (5 groups)
    for i in range(max(len(a), len(b))):
        if i < len(a):
            order.append(a[i])
        if i < len(b):
            order.append(b[i])
    # one partial slot per group, laid out per window
    base = {}
    off = 0
    for wi in range(4):
        base[wi] = off
        off += len(per_w[wi])
    plan = []
    cur = dict((wi, base[wi]) for wi in range(4))
    for (wi, gi, l0, n) in order:
        dve = (wi, gi) in DVE_GROUPS
        plan.append((wi, gi, l0, n, cur[wi], dve))
        cur[wi] += 1
    slab_bounds = {wi: (base[wi], cur[wi]) for wi in range(4)}
    return plan, slab_bounds, off


# ---------------------------------------------------------------- host prep
def prep_weights(char_vectors, filts, w_proj, hw_ws, hw_bs):
    units, _ = build_schedule()
    U = len(units)
    wconv = np.zeros((128, U, 128), np.float32)
    for key, u in units.items():
        kind, w = key[0], key[1]
        f = filts[w].reshape(NF, w, D)  # [100, w, 64]
        if kind == "pair":
            j = key[2]
            wconv[0:64, u, 0:NF] = f[:, j, :].T
            wconv[64:128, u, 0:NF] = f[:, j + 1, :].T
        else:
            h = key[2]
            wconv[64 * h:64 * h + 64, u, 0:NF] = f[:, w - 1, :].T
    wproj = np.zeros((128, 4, 128), np.float32)
    for c in range(4):
        wproj[0:NF, c, :] = w_proj[:, c * NF:(c + 1) * NF].T
    whw = np.zeros((128, 4, 128), np.float32)
    for i, wm in enumerate(hw_ws):  # [t_w0, g_w0, t_w1, g_w1]
        whw[:, i, :] = wm.T
    bias = np.zeros((128, 4), np.float32)
    for i, bv in enumerate(hw_bs):  # [t_b0, g_b0, t_b1, g_b1]
        bias[:, i] = bv
    return {
        "cv": np.ascontiguousarray(char_vectors.astype(BF16)),
        "wconv": np.ascontiguousarray(wconv.astype(BF16)),
        "wproj": np.ascontiguousarray(wproj.astype(BF16)),
        "whw": np.ascontiguousarray(whw.astype(BF16)),
        "bias": np.ascontiguousarray(bias),
    }


# ---------------------------------------------------------------- program
def build_program(n_tiles=N_TILES, repeat=1):
    from concourse import bacc
    import concourse.mybir as mybir
    from concourse.tile import TileContext

    dt = mybir.dt
    AF = mybir.ActivationFunctionType
    AL = mybir.AluOpType
    units, sched = build_schedule()
    U = len(units)
    plan, slab_bounds, n_slab = conv_plan()

    nc = bacc.Bacc("TRN2", target_bir_lowering=False, debug=False, num_devices=N_CORES)

    idx_d = nc.dram_tensor("idx", [16, TOK_PER_CORE], dt.int16, kind="ExternalInput")
    cv_d = nc.dram_tensor("cv", [VOCAB, D], dt.bfloat16, kind="ExternalInput")
    wconv_d = nc.dram_tensor("wconv", [128, U, 128], dt.bfloat16, kind="ExternalInput")
    wproj_d = nc.dram_tensor("wproj", [128, 4, 128], dt.bfloat16, kind="ExternalInput")
    whw_d = nc.dram_tensor("whw", [128, 4, 128], dt.bfloat16, kind="ExternalInput")
    bias_d = nc.dram_tensor("bias", [128, 4], dt.float32, kind="ExternalInput")
    out_d = nc.dram_tensor("out", [128, TOK_PER_CORE], dt.float32, kind="ExternalOutput")

    with TileContext(nc) as tc:
        with (
            tc.tile_pool(name="const", bufs=1) as cpool,
            tc.tile_pool(name="io", bufs=2) as iopool,
            tc.tile_pool(name="work", bufs=2) as wpool,
            tc.tile_pool(name="slab", bufs=1) as spool,
            tc.tile_pool(name="grp_psum", bufs=2, space="PSUM") as gpsum,
            tc.tile_pool(name="vec_psum", bufs=2, space="PSUM") as vpsum,
        ):
            cv_sb = cpool.tile([VOCAB, D], dt.bfloat16)
            nc.sync.dma_start(cv_sb, cv_d.ap())
            wconv_sb = cpool.tile([128, U, 128], dt.bfloat16)
            nc.sync.dma_start(wconv_sb, wconv_d.ap())
            wproj_sb = cpool.tile([128, 4, 128], dt.bfloat16)
            nc.sync.dma_start(wproj_sb, wproj_d.ap())
            whw_sb = cpool.tile([128, 4, 128], dt.bfloat16)
            nc.sync.dma_start(whw_sb, whw_d.ap())
            bias_sb = cpool.tile([128, 4], dt.float32)
            nc.sync.dma_start(bias_sb, bias_d.ap())
            iota_i = cpool.tile([VOCAB, 1], dt.int32)
            nc.gpsimd.iota(iota_i, pattern=[[1, 1]], base=0, channel_multiplier=1)
            iota_sb = cpool.tile([VOCAB, 1], dt.float32)
            nc.vector.tensor_copy(iota_sb, iota_i)

            def prep_esb(ti, first):
                """one-hot + embed matmuls + extracts + Pool E1 copies for
                tile ti; returns the e_sb tile."""
                tok0, Tc = TILES[ti]
                idx_b = iopool.tile([VOCAB, 16, T], dt.int16, tag="idxb")
                for h0 in (0, 8):
                    nc.sync.dma_start(
                        idx_b[:, h0:h0 + 8, 0:Tc],
                        idx_d.ap()[h0:h0 + 8, tok0:tok0 + Tc]
                        .partition_broadcast(VOCAB),
                    )
                oh = iopool.tile([VOCAB, 16, T], dt.bfloat16, tag="oh")
                for h0 in (0, 8):
                    nc.vector.tensor_scalar(
                        oh[:, h0:h0 + 8, 0:Tc], idx_b[:, h0:h0 + 8, 0:Tc],
                        iota_sb[:, :], None, AL.is_equal)
                e_sb = iopool.tile([128, 15, T], dt.bfloat16, tag="esb")
                for r0 in (0, 3, 6):
                    n = min(GRP, 8 - r0)
                    g = gpsum.tile([128, GRP, 512], dt.float32, tag="grp")
                    for i in range(n):
                        p = r0 + i
                        nc.tensor.matmul(g[0:64, i, 0:Tc], cv_sb, oh[:, 2 * p, 0:Tc],
                                         start=True, stop=True, tile_position=(0, 0))
                        nc.tensor.matmul(g[64:128, i, 0:Tc], cv_sb,
                                         oh[:, 2 * p + 1, 0:Tc],
                                         start=True, stop=True, tile_position=(0, 64))
                    nc.scalar.copy(e_sb[:, r0:r0 + n, 0:Tc], g[:, 0:n, 0:Tc])
                # E1 shifted pairs on DVE (4x tensor_copy, ~1us each)
                nc.vector.tensor_copy(e_sb[0:64, 8:15, 0:Tc],
                                      e_sb[64:128, 0:7, 0:Tc])
                nc.vector.tensor_copy(e_sb[64:128, 8:15, 0:Tc],
                                      e_sb[0:64, 1:8, 0:Tc])
                return e_sb

            for _rep in range(repeat):
                e_next = prep_esb(0, True)
                for ti in range(n_tiles):
                    tok0, Tc = TILES[ti]
                    e_sb = e_next
                    pw = spool.tile([128, n_slab, Tc], dt.bfloat16, tag="pw",
                                    bufs=2)
                    mfin = wpool.tile([128, 4, T], dt.bfloat16, tag="m4")
                    th = wpool.tile([128, 4, T], dt.bfloat16, tag="th")

                    done = {wi: 0 for wi in range(4)}
                    n_groups = {wi: 0 for wi in range(4)}
                    for (wi, gi, l0, n, slot, dve) in plan:
                        n_groups[wi] += 1
                    prep_emitted = False

                    for pi, (wi, gi, l0, n, slot, dve) in enumerate(plan):
                        w, wl = sched[wi]
                        g = gpsum.tile([128, GRP, 512], dt.float32, tag="grp")
                        for li in range(n):
                            ops = wl[l0 + li]
                            for oi, (u, sl) in enumerate(ops):
                                nc.tensor.matmul(
                                    g[:, li, 0:Tc], wconv_sb[:, u, :],
                                    e_sb[:, sl, 0:Tc],
                                    start=(oi == 0), stop=(oi == len(ops) - 1),
                                )
                        if dve:
                            if n == 1:
                                nc.vector.tensor_copy(pw[:, slot, :],
                                                      g[:, 0, 0:Tc])
                            else:
                                nc.vector.tensor_reduce(
                                    pw[:, slot, :],
                                    g[:, 0:n, 0:Tc].rearrange("p g t -> p t g"),
                                    axis=mybir.AxisListType.X, op=AL.max)
                        else:
                            # extract to scratch, fold 3->1 into the partial
                            if n == 1:
                                nc.scalar.copy(pw[:, slot, :], g[:, 0, 0:Tc])
                            else:
                                px = wpool.tile([128, GRP, Tc], dt.bfloat16,
                                                tag="px", bufs=3)
                                nc.scalar.copy(px[:, 0:n, :], g[:, 0:n, 0:Tc])
                                if n == 2:
                                    nc.vector.tensor_max(pw[:, slot, :],
                                                         px[:, 0, :], px[:, 1, :])
                                else:
                                    sx = wpool.tile([128, Tc], dt.bfloat16,
                                                    tag="sx", bufs=3)
                                    nc.vector.tensor_max(sx, px[:, 0, :],
                                                         px[:, 1, :])
                                    nc.vector.tensor_max(pw[:, slot, :], sx,
                                                         px[:, 2, :])
                        done[wi] += 1
                        # shallow tree over this window's partials once all
                        # its groups drained; tanh per window right after
                        if done[wi] == n_groups[wi]:
                            lo, hi = slab_bounds[wi]
                            items = [pw[:, s, :] for s in range(lo, hi)]
                            while len(items) > 2:
                                nx = []
                                for k in range(0, len(items) - 1, 2):
                                    tz = wpool.tile([128, Tc], dt.bfloat16,
                                                    tag="tz", bufs=4)
                                    nc.vector.tensor_max(tz, items[k],
                                                         items[k + 1])
                                    nx.append(tz)
                                if len(items) % 2:
                                    nx.append(items[-1])
                                items = nx
                            nc.vector.tensor_max(mfin[:, wi, 0:Tc], items[0],
                                                 items[1])
                            nc.scalar.activation(th[:, wi, 0:Tc],
                                                 mfin[:, wi, 0:Tc], AF.Tanh)
                        # emit next tile's embedding prep early so its
                        # extracts + E1 overlap this tile's conv
                        if pi == 3 and ti + 1 < n_tiles:
                            e_next = prep_esb(ti + 1, False)
                            prep_emitted = True
                    if not prep_emitted and ti + 1 < n_tiles:
                        e_next = prep_esb(ti + 1, False)

                    # ---- projection
                    x_ps = vpsum.tile([128, 512], dt.float32, tag="vec")
                    for c in range(4):
                        nc.tensor.matmul(x_ps[:, 0:Tc], wproj_sb[:, c, :],
                                         th[:, c, 0:Tc], start=(c == 0), stop=(c == 3))
                    xs = wpool.tile([128, T], dt.bfloat16, tag="xs")
                    nc.scalar.copy(xs[:, 0:Tc], x_ps[:, 0:Tc])

                    # ---- highway x2
                    for hl in range(2):
                        t_ps = vpsum.tile([128, 512], dt.float32, tag="vec")
                        g_ps = vpsum.tile([128, 512], dt.float32, tag="vec")
                        nc.tensor.matmul(t_ps[:, 0:Tc], whw_sb[:, 2 * hl, :],
                                         xs[:, 0:Tc], start=True, stop=True)
                        nc.tensor.matmul(g_ps[:, 0:Tc], whw_sb[:, 2 * hl + 1, :],
                                         xs[:, 0:Tc], start=True, stop=True)
                        tt = wpool.tile([128, T], dt.bfloat16, tag="tt")
                        gg = wpool.tile([128, T], dt.bfloat16, tag="gg")
                        nc.scalar.activation(tt[:, 0:Tc], t_ps[:, 0:Tc], AF.Relu,
                                             bias=bias_sb[:, 2 * hl:2 * hl + 1],
                                             scale=1.0)
                        nc.scalar.activation(gg[:, 0:Tc], g_ps[:, 0:Tc], AF.Sigmoid,
                                             bias=bias_sb[:, 2 * hl + 1:2 * hl + 2],
                                             scale=1.0)
                        dd = wpool.tile([128, T], dt.bfloat16, tag="dd")
                        gd = wpool.tile([128, T], dt.bfloat16, tag="gd")
                        nc.vector.tensor_sub(dd[:, 0:Tc], tt[:, 0:Tc], xs[:, 0:Tc])
                        nc.vector.tensor_mul(gd[:, 0:Tc], gg[:, 0:Tc], dd[:, 0:Tc])
                        if hl == 0:
                            xs2 = wpool.tile([128, T], dt.bfloat16, tag="xs")
                            nc.vector.tensor_add(xs2[:, 0:Tc], xs[:, 0:Tc],
                                                 gd[:, 0:Tc])
                            xs = xs2
                        else:
                            xf = wpool.tile([128, T], dt.float32, tag="xf")
                            nc.vector.tensor_add(xf[:, 0:Tc], xs[:, 0:Tc],
                                                 gd[:, 0:Tc])
                            nc.sync.dma_start(out_d.ap()[:, tok0:tok0 + Tc],
                                              xf[:, 0:Tc])

    nc.compile()
    return nc


# ---------------------------------------------------------------- runner
def _make_sharded(nc):
    import jax
    from jax.sharding import Mesh, PartitionSpec
    from jax.experimental.shard_map import shard_map
    from concourse import bass2jax, mybir

    bass2jax.install_neuronx_cc_hook()
    partition_name = nc.partition_id_tensor.name if nc.partition_id_tensor else None
    in_names, out_names, out_avals = [], [], []
    for alloc in nc.m.functions[0].allocations:
        if not isinstance(alloc, mybir.MemoryLocationSet):
            continue
        name = alloc.memorylocations[0].name
        if alloc.kind == "ExternalInput":
            if name != partition_name:
                in_names.append(name)
        elif alloc.kind == "ExternalOutput":
            out_names.append(name)
            out_avals.append(
                jax.core.ShapedArray(tuple(alloc.tensor_shape), mybir.dt.np(alloc.dtype))
            )
    n_params = len(in_names)
    all_in_names = in_names + out_names
    if partition_name is not None:
        all_in_names = all_in_names + [partition_name]

    def _body(*args):
        operands = list(args)
        if partition_name is not None:
            operands.append(bass2jax.partition_id_tensor())
        outs = bass2jax._bass_exec_p.bind(
            *operands,
            out_avals=tuple(out_avals),
            in_names=tuple(all_in_names),
            out_names=tuple(out_names),
            lowering_input_output_aliases=(),
            sim_require_finite=True,
            sim_require_nnan=True,
            nc=nc,
        )
        return tuple(outs)

    devices = jax.devices()[:N_CORES]
    mesh = Mesh(np.asarray(devices), ("core",))
    n_outs = len(out_names)
    in_specs = (PartitionSpec("core"),) * (n_params + n_outs)
    out_specs = (PartitionSpec("core"),) * n_outs
    fn = jax.jit(
        shard_map(_body, mesh=mesh, in_specs=in_specs, out_specs=out_specs,
                  check_rep=False),
        keep_unused=True,
    )
    meta = {"in_names": in_names, "out_names": out_names, "out_avals": out_avals,
            "n_params": n_params}
    return fn, meta


def _get_runner():
    if "runner" not in _cache:
        nc = build_program()
        _cache["nc"] = nc
        _cache["runner"] = _make_sharded(nc)
    return _cache["runner"]


def _concat_inputs(in_maps, meta):
    concat_in = [
        np.concatenate([in_maps[c][name] for c in range(N_CORES)], axis=0)
        for name in meta["in_names"]
    ]
    concat_zeros = [
        np.zeros((N_CORES * a.shape[0], *a.shape[1:]), a.dtype)
        for a in meta["out_avals"]
    ]
    return concat_in, concat_zeros


def make_in_maps(char_idxs, char_vectors, filt2, filt3, filt4, filt5, w_proj,
                 t_w0, t_b0, t_w1, t_b1, g_w0, g_b0, g_w1, g_b1):
    wts = prep_weights(
        np.asarray(char_vectors, np.float32),
        {2: np.asarray(filt2, np.float32), 3: np.asarray(filt3, np.float32),
         4: np.asarray(filt4, np.float32), 5: np.asarray(filt5, np.float32)},
        np.asarray(w_proj, np.float32),
        [np.asarray(t_w0, np.float32), np.asarray(g_w0, np.float32),
         np.asarray(t_w1, np.float32), np.asarray(g_w1, np.float32)],
        [np.asarray(t_b0, np.float32), np.asarray(g_b0, np.float32),
         np.asarray(t_b1, np.float32), np.asarray(g_b1, np.float32)],
    )
    idx = np.asarray(char_idxs)
    assert idx.shape == (B, S, W)
    rows_per_core = B // N_CORES
    in_maps = []
    for c in range(N_CORES):
        m = dict(wts)
        m["idx"] = np.ascontiguousarray(
            idx[c * rows_per_core:(c + 1) * rows_per_core]
            .reshape(TOK_PER_CORE, 16).T.astype(np.int16)
        )
        in_maps.append(m)
    return in_maps


def kernel(**inputs) -> np.ndarray:
    in_maps = make_in_maps(**inputs)
    sharded, meta = _get_runner()
    concat_in, concat_zeros = _concat_inputs(in_maps, meta)
    out_arrs = sharded(*concat_in, *concat_zeros)
    out = np.asarray(out_arrs[0])  # [8*128, 3200]
    rows_per_core = B // N_CORES
    parts = []
    for c in range(N_CORES):
        oc = out[c * 128:(c + 1) * 128]  # [128, 3200]
        parts.append(oc.T.reshape(rows_per_core, S, HID))
    return np.ascontiguousarray(np.concatenate(parts, axis=0))


def time_kernel(inputs, repeat=(8, 25), reps=20):
    """Per-pass exec time from the slope between two repeat factors."""
    import time
    import jax
    from jax.sharding import Mesh, PartitionSpec, NamedSharding

    in_maps = make_in_maps(**inputs)
    sharded, meta = _get_runner()
    concat_in, concat_zeros = _concat_inputs(in_maps, meta)
    mesh = Mesh(np.asarray(jax.devices()[:N_CORES]), ("core",))
    shd = NamedSharding(mesh, PartitionSpec("core"))
    d_in = [jax.device_put(a, shd) for a in concat_in]
    d_zero = [jax.device_put(a, shd) for a in concat_zeros]

    r1, r2 = repeat
    fns = []
    for r in (r1, r2):
        key = ("rep", r)
        if key not in _cache:
            nc_r = build_program(repeat=r)
            _cache[key] = _make_sharded(nc_r)
        fns.append(_cache[key][0])
    fn_1, fn_2 = fns

    def timed(fn, args):
        t0 = time.perf_counter()
        out = fn(*args)
        jax.block_until_ready(out)
        return time.perf_counter() - t0

    timed(fn_1, (*d_in, *d_zero))
    timed(fn_2, (*d_in, *d_zero))
    diffs, t1s = [], []
    for _ in range(reps):
        a = timed(fn_1, (*d_in, *d_zero))
        b = timed(fn_2, (*d_in, *d_zero))
        t1s.append(a)
        diffs.append(b - a)
    diffs.sort()
    t1s.sort()
    med = diffs[len(diffs) // 2]
    per_pass = med / (r2 - r1)
    return per_pass * 1e9, t1s[len(t1s) // 2] * 1e9, med * 1e9
